# revision 5
# baseline (speedup 1.0000x reference)
"""Causal self-attention (RoPE + QK-RMSNorm, GQA 16q/8kv) Trainium2 Bass kernel.

Sharding: 8 cores = 2 batch x 4 tensor-parallel. Core c handles batch b=c//4 and
q-heads [4*tp, 4*tp+4), kv-heads [2*tp, 2*tp+2) where tp=c%4. Each core returns a
partial (T, C) output = O_heads @ wo[rows of its heads]; host sums the 4 partials
per batch (the "all-reduce after c_proj").

All matmuls run as float32r (full-rate fp32 mode, ~1e-4 rel err).
"""
import sys
import math

sys.path.insert(0, "/opt/trn_rl_repo")

import numpy as np
import concourse.bacc as bacc
import concourse.mybir as mybir
import concourse.tile as tile
from concourse.bass_utils import run_bass_kernel_spmd

P = 128
T = 2048
C = 2048
KO = C // P          # 16 contraction tiles
D = 128              # head dim
NQ = 4               # q heads per core
NK = 2               # kv heads per core
NF = NQ + NK         # 6 rope/rms feature blocks (4 q + 2 k)
FQ = NQ * D          # 512
FK = NK * D          # 256
TCH = 512            # phase-1 T-chunk
NCHUNK = T // TCH    # 4
SPAN = 512           # attention q-span
NSPAN = T // SPAN    # 4
KB = T // P          # 16 key blocks
SCALE = 1.0 / math.sqrt(D)
EPS = 1.1920929e-07

f32 = mybir.dt.float32
f32r = mybir.dt.float32r

AF = mybir.ActivationFunctionType


def build():
    nc = bacc.Bacc("TRN2", target_bir_lowering=False)
    xT = nc.dram_tensor("xT", (C, T), f32, kind="ExternalInput")
    wq = nc.dram_tensor("wq", (C, FQ), f32, kind="ExternalInput")
    wk = nc.dram_tensor("wk", (C, FK), f32, kind="ExternalInput")
    wv = nc.dram_tensor("wv", (C, FK), f32, kind="ExternalInput")
    wo = nc.dram_tensor("wo", (FQ, C), f32, kind="ExternalInput")
    cc = nc.dram_tensor("cc", (P, T), f32, kind="ExternalInput")    # [cos; cos]
    ss = nc.dram_tensor("ss", (P, T), f32, kind="ExternalInput")    # [sin; -sin]
    maskT = nc.dram_tensor("maskT", (P, 4, SPAN), f32, kind="ExternalInput")
    ident = nc.dram_tensor("ident", (P, P), f32, kind="ExternalInput")
    y = nc.dram_tensor("y", (T, C), f32, kind="ExternalOutput")

    xT_r = xT.rearrange("(ko p) t -> p ko t", p=P)
    wq_r = wq.rearrange("(ko p) f -> p ko f", p=P)
    wk_r = wk.rearrange("(ko p) f -> p ko f", p=P)
    wv_r = wv.rearrange("(ko p) f -> p ko f", p=P)
    wo_r = wo.rearrange("(ko p) n -> p ko n", p=P)

    with tile.TileContext(nc) as tc:
        with tc.tile_pool(name="persist", bufs=1) as persist:
            # persistent across phases
            qk_rt = persist.tile([P, NF, T], f32r, tag="qk_rt")   # roped+normed qT/kT, 6MB
            v_sb = persist.tile([P, KB, FK], f32r, tag="v_sb")    # V natural [t-part, kb, kv-feat], 2MB
            cc_sb = persist.tile([P, T], f32, tag="cc_sb")
            ss_sb = persist.tile([P, T], f32, tag="ss_sb")
            id_sb = persist.tile([P, P], f32, tag="id_sb")
            ones_col = persist.tile([P, 1], f32r, tag="ones_col")
            ones_row = persist.tile([1, P], f32r, tag="ones_row")
            eps_sb = persist.tile([1, 1], f32, tag="eps_sb")
            nc.vector.memset(eps_sb[:], EPS)
            nc.sync.dma_start(cc_sb[:], cc[:, :])
            nc.sync.dma_start(ss_sb[:], ss[:, :])
            nc.sync.dma_start(id_sb[:], ident[:, :])
            ones_f32 = persist.tile([P, 1], f32, tag="ones_f32")
            ones_row_f32 = persist.tile([1, P], f32, tag="ones_row_f32")
            nc.vector.memset(ones_f32[:], 1.0)
            nc.vector.memset(ones_row_f32[:], 1.0)
            nc.vector.tensor_copy(ones_col[:], ones_f32[:])
            nc.vector.tensor_copy(ones_row[:], ones_row_f32[:])

            # ---------------- Phase 1: QKV projections + RoPE + V transpose ----------------
            with (
                tc.tile_pool(name="ph1w", bufs=1) as wpool,
                tc.tile_pool(name="ph1x", bufs=1) as xpool,
                tc.tile_pool(name="ph1t", bufs=3) as tpool,
                tc.tile_pool(name="ph1ps", bufs=2, space="PSUM") as ps1,
            ):
                wq_sb = wpool.tile([P, KO, FQ], f32r, tag="wq_sb")
                wk_sb = wpool.tile([P, KO, FK], f32r, tag="wk_sb")
                wv_sb = wpool.tile([P, KO, FK], f32r, tag="wv_sb")
                nc.sync.dma_start(wq_sb[:], wq_r.bitcast(f32r))
                nc.sync.dma_start(wk_sb[:], wk_r.bitcast(f32r))
                nc.sync.dma_start(wv_sb[:], wv_r.bitcast(f32r))

                for tch in range(NCHUNK):
                    t0 = tch * TCH
                    xt = xpool.tile([P, KO, TCH], f32r, tag="xt")
                    # per-ko DMAs so matmuls can start as slices land
                    for ko in range(KO):
                        nc.sync.dma_start(
                            xt[:, ko, :], xT_r[:, ko, t0 : t0 + TCH].bitcast(f32r)
                        )
                    # qT / kT feature blocks (4 q heads + 2 k heads)
                    for fb in range(NF):
                        if fb < NQ:
                            w_ap = wq_sb[:, :, fb * D : (fb + 1) * D]
                        else:
                            w_ap = wk_sb[:, :, (fb - NQ) * D : (fb - NQ + 1) * D]
                        pqk = ps1.tile([P, TCH], f32, tag="ps_qk")
                        for ko in range(KO):
                            nc.tensor.matmul(
                                pqk[:], w_ap[:, ko], xt[:, ko, :],
                                start=(ko == 0), stop=(ko == KO - 1),
                            )
                        # rope: raw chunk + half-swapped chunk
                        raw = tpool.tile([P, TCH], f32, tag="rope_raw")
                        nc.vector.tensor_copy(raw[:], pqk[:])
                        swp = tpool.tile([P, TCH], f32, tag="rope_swp")
                        nc.sync.dma_start(swp[0:64, :], raw[64:128, :])
                        nc.sync.dma_start(swp[64:128, :], raw[0:64, :])
                        tmp = tpool.tile([P, TCH], f32, tag="rope_tmp")
                        dst = qk_rt[:, fb, t0 : t0 + TCH]
                        nc.vector.tensor_mul(dst, raw[:], cc_sb[:, t0 : t0 + TCH])
                        nc.vector.tensor_mul(tmp[:], swp[:], ss_sb[:, t0 : t0 + TCH])
                        nc.vector.tensor_add(dst, dst, tmp[:])
                    # vT blocks -> transpose -> V natural
                    for vfb in range(NK):
                        w_ap = wv_sb[:, :, vfb * D : (vfb + 1) * D]
                        pvt = ps1.tile([P, TCH], f32, tag="ps_vt")
                        for ko in range(KO):
                            nc.tensor.matmul(
                                pvt[:], w_ap[:, ko], xt[:, ko, :],
                                start=(ko == 0), stop=(ko == KO - 1),
                            )
                        vt_sb = tpool.tile([P, TCH], f32, tag="vt_sb")
                        nc.vector.tensor_copy(vt_sb[:], pvt[:])
                        for tb in range(TCH // P):
                            ptr = ps1.tile([P, P], f32, tag="ps_tr")
                            nc.tensor.transpose(
                                ptr[:], vt_sb[:, tb * P : (tb + 1) * P], id_sb[:]
                            )
                            nc.vector.tensor_copy(
                                v_sb[:, tch * (TCH // P) + tb, vfb * D : (vfb + 1) * D],
                                ptr[:],
                            )

            # ---------------- Phase 2: QK RMS norm (over head dim = partitions) ----------
            with (
                tc.tile_pool(name="ph2t", bufs=3) as p2,
                tc.tile_pool(name="ph2ps", bufs=2, space="PSUM") as ps2,
            ):
                for fb in range(NF):
                    for s in range(NSPAN):
                        c0 = s * SPAN
                        seg = qk_rt[:, fb, c0 : c0 + SPAN]
                        sq = p2.tile([P, SPAN], f32r, tag="sq")
                        nc.vector.tensor_mul(sq[:], seg, seg)
                        pms = ps2.tile([1, SPAN], f32, tag="ps_ms")
                        nc.tensor.matmul(pms[:], ones_col[:], sq[:], start=True, stop=True)
                        srt = p2.tile([1, SPAN], f32, tag="srt")
                        nc.scalar.activation(srt[:], pms[:], AF.Sqrt, bias=eps_sb[:], scale=1.0 / D)
                        rstd = p2.tile([1, SPAN], f32r, tag="rstd")
                        with nc.allow_low_precision(reason="fp32r rounding for matmul rhs"):
                            nc.vector.reciprocal(rstd[:], srt[:])
                        pb = ps2.tile([P, SPAN], f32, tag="ps_b")
                        nc.tensor.matmul(pb[:], ones_row[:], rstd[:], start=True, stop=True)
                        nc.vector.tensor_mul(seg, seg, pb[:])

            # ---------------- Phase 3: attention + Phase 4: output projection ------------
            with (
                tc.tile_pool(name="ph3s", bufs=1) as p3s,
                tc.tile_pool(name="ph3t", bufs=4) as p3,
            ):
                ot_sb = p3s.tile([P, NQ, T], f32r, tag="ot_sb")
                mask_sb = p3s.tile([P, 4, SPAN], f32, tag="mask_sb")
                wo_sb = p3s.tile([P, NQ, C], f32r, tag="wo_sb")
                nc.sync.dma_start(mask_sb[:], maskT[:, :, :])
                nc.sync.dma_start(wo_sb[:], wo_r.bitcast(f32r))

                with (
                    tc.tile_pool(name="ph3ps", bufs=2, space="PSUM") as ps3,
                    tc.tile_pool(name="ph3ot", bufs=1, space="PSUM") as psot,
                ):
                  for h in range(NQ):
                    j = h // 2
                    for s in range(NSPAN):
                        q0 = s * SPAN
                        nkb = 4 * s + 4
                        ot_ps = psot.tile([P, SPAN], f32, tag="ot_ps")
                        sum_ps = psot.tile([1, SPAN], f32, tag="sum_ps")
                        q_ap = qk_rt[:, h, q0 : q0 + SPAN]
                        for kb in range(nkb):
                            st_ps = ps3.tile([P, SPAN], f32, tag="st_ps")
                            nc.tensor.matmul(
                                st_ps[:],
                                qk_rt[:, NQ + j, kb * P : (kb + 1) * P],
                                q_ap,
                                start=True, stop=True,
                            )
                            pt = p3.tile([P, SPAN], f32r, tag="pt")
                            nc.scalar.activation(pt[:], st_ps[:], AF.Exp, scale=SCALE)
                            if kb >= 4 * s:
                                nc.vector.tensor_mul(
                                    pt[:], pt[:], mask_sb[:, kb - 4 * s, :]
                                )
                            nc.tensor.matmul(
                                ot_ps[:],
                                v_sb[:, kb, j * D : (j + 1) * D],
                                pt[:],
                                start=(kb == 0), stop=(kb == nkb - 1),
                                skip_group_check=True,
                            )
                            nc.tensor.matmul(
                                sum_ps[:],
                                ones_col[:],
                                pt[:],
                                start=(kb == 0), stop=(kb == nkb - 1),
                                skip_group_check=True,
                            )
                        rec = p3.tile([1, SPAN], f32r, tag="rec")
                        with nc.allow_low_precision(reason="fp32r rounding for matmul rhs"):
                            nc.vector.reciprocal(rec[:], sum_ps[:])
                        bc_ps = ps3.tile([P, SPAN], f32, tag="bc_ps")
                        nc.tensor.matmul(bc_ps[:], ones_row[:], rec[:], start=True, stop=True)
                        bc_sb = p3.tile([P, SPAN], f32, tag="bc_sb")
                        nc.scalar.activation(bc_sb[:], bc_ps[:], AF.Copy)
                        nc.vector.tensor_mul(
                            ot_sb[:, h, q0 : q0 + SPAN], ot_ps[:], bc_sb[:]
                        )

                # output projection: y[tb, n] += OT.T @ wo_rows
                with (
                    tc.tile_pool(name="ph4t", bufs=3) as p4,
                    tc.tile_pool(name="ph4ps", bufs=3, space="PSUM") as ps4,
                ):
                    for tb in range(T // P):
                        for nch in range(C // 512):
                            yps = ps4.tile([P, 512], f32, tag="yps")
                            for h in range(NQ):
                                nc.tensor.matmul(
                                    yps[:],
                                    ot_sb[:, h, tb * P : (tb + 1) * P],
                                    wo_sb[:, h, nch * 512 : (nch + 1) * 512],
                                    start=(h == 0), stop=(h == NQ - 1),
                                )
                            ysb = p4.tile([P, 512], f32, tag="ysb")
                            nc.vector.tensor_copy(ysb[:], yps[:])
                            nc.sync.dma_start(
                                y[tb * P : (tb + 1) * P, nch * 512 : (nch + 1) * 512],
                                ysb[:],
                            )
    nc.compile()
    return nc


_NC_CACHE = None


def _get_nc():
    global _NC_CACHE
    if _NC_CACHE is None:
        _NC_CACHE = build()
    return _NC_CACHE


def _host_inputs(x, cos, sin, wq, wk, wv, wo):
    """Build the 8 per-core input maps."""
    cosT = np.ascontiguousarray(cos[0, :, 0, :].T).astype(np.float32)  # (64, T)
    sinT = np.ascontiguousarray(sin[0, :, 0, :].T).astype(np.float32)
    cc = np.concatenate([cosT, cosT], axis=0)          # (128, T)
    ss = np.concatenate([sinT, -sinT], axis=0)
    # maskT[r][k, q] = 1 if q >= 128*r + k  (within a 512-q span, k-block offset r)
    qidx = np.arange(SPAN)[None, None, :]
    kidx = np.arange(P)[:, None, None]
    ridx = np.arange(4)[None, :, None]
    maskT = (qidx >= P * ridx + kidx).astype(np.float32)  # (128, 4, 512)
    ident = np.eye(P, dtype=np.float32)

    xTs = [np.ascontiguousarray(x[b].T) for b in range(2)]
    in_maps = []
    for c in range(8):
        b, tp = divmod(c, 4)
        in_maps.append(
            {
                "xT": xTs[b],
                "wq": np.ascontiguousarray(wq[:, tp * FQ : (tp + 1) * FQ]),
                "wk": np.ascontiguousarray(wk[:, tp * FK : (tp + 1) * FK]),
                "wv": np.ascontiguousarray(wv[:, tp * FK : (tp + 1) * FK]),
                "wo": np.ascontiguousarray(wo[tp * FQ : (tp + 1) * FQ, :]),
                "cc": cc,
                "ss": ss,
                "maskT": maskT,
                "ident": ident,
            }
        )
    return in_maps


def kernel(x, cos, sin, wq, wk, wv, wo, trace=False):
    x = np.asarray(x, dtype=np.float32)
    cos = np.asarray(cos, dtype=np.float32)
    sin = np.asarray(sin, dtype=np.float32)
    wq = np.asarray(wq, dtype=np.float32)
    wk = np.asarray(wk, dtype=np.float32)
    wv = np.asarray(wv, dtype=np.float32)
    wo = np.asarray(wo, dtype=np.float32)

    nc = _get_nc()
    in_maps = _host_inputs(x, cos, sin, wq, wk, wv, wo)
    res = run_bass_kernel_spmd(nc, in_maps, core_ids=list(range(8)), trace=trace)
    out = np.zeros((2, T, C), dtype=np.float32)
    for c in range(8):
        b = c // 4
        out[b] += res.results[c]["y"]
    if trace:
        return out, res
    return out


# revision 8
# speedup vs baseline: 1.0582x; 1.0582x over previous
"""Causal self-attention (RoPE + QK-RMSNorm, GQA 16q/8kv) Trainium2 Bass kernel.

Sharding: 8 cores = 2 batch x 4 tensor-parallel. Core c handles batch b=c//4 and
q-heads [4*tp, 4*tp+4), kv-heads [2*tp, 2*tp+2) where tp=c%4. Each core returns a
partial (T, C) output = O_heads @ wo[rows of its heads]; host sums the 4 partials
per batch (the "all-reduce after c_proj").

Matmuls run in bf16 (fp32 PSUM accumulation); softmax row-sum normalization and
RMS statistics stay in fp32/fp32r.
"""
import sys
import math

sys.path.insert(0, "/opt/trn_rl_repo")

import numpy as np
import ml_dtypes
import concourse.bacc as bacc
import concourse.mybir as mybir
import concourse.tile as tile
from concourse.bass_utils import run_bass_kernel_spmd

P = 128
T = 2048
C = 2048
KO = C // P          # 16 contraction tiles
D = 128              # head dim
NQ = 4               # q heads per core
NK = 2               # kv heads per core
NF = NQ + NK         # 6 rope/rms feature blocks (4 q + 2 k)
FQ = NQ * D          # 512
FK = NK * D          # 256
TCH = 512            # phase-1 T-chunk
NCHUNK = T // TCH    # 4
SPAN = 512           # attention q-span
NSPAN = T // SPAN    # 4
KB = T // P          # 16 key blocks
SCALE = 1.0 / math.sqrt(D)
EPS = 1.1920929e-07

f32 = mybir.dt.float32
f32r = mybir.dt.float32r
bf16 = mybir.dt.bfloat16

AF = mybir.ActivationFunctionType


def build():
    nc = bacc.Bacc("TRN2", target_bir_lowering=False)
    xT = nc.dram_tensor("xT", (C, T), bf16, kind="ExternalInput")
    wq = nc.dram_tensor("wq", (C, FQ), bf16, kind="ExternalInput")
    wk = nc.dram_tensor("wk", (C, FK), bf16, kind="ExternalInput")
    wv = nc.dram_tensor("wv", (C, FK), bf16, kind="ExternalInput")
    wo = nc.dram_tensor("wo", (FQ, C), bf16, kind="ExternalInput")
    cc = nc.dram_tensor("cc", (P, T), f32, kind="ExternalInput")    # [cos; cos]
    ss = nc.dram_tensor("ss", (P, T), f32, kind="ExternalInput")    # [sin; -sin]
    maskT = nc.dram_tensor("maskT", (P, 4, SPAN), f32, kind="ExternalInput")
    ident = nc.dram_tensor("ident", (P, P), bf16, kind="ExternalInput")
    y = nc.dram_tensor("y", (T, C), f32, kind="ExternalOutput")

    xT_r = xT.rearrange("(ko p) t -> p ko t", p=P)
    wq_r = wq.rearrange("(ko p) f -> p ko f", p=P)
    wk_r = wk.rearrange("(ko p) f -> p ko f", p=P)
    wv_r = wv.rearrange("(ko p) f -> p ko f", p=P)
    wo_r = wo.rearrange("(ko p) n -> p ko n", p=P)

    with tile.TileContext(nc) as tc:
        with tc.tile_pool(name="persist", bufs=1) as persist:
            # persistent across phases
            qk_rt = persist.tile([P, NF, T], bf16, tag="qk_rt")   # roped+normed qT/kT
            v_sb = persist.tile([P, KB, FK], bf16, tag="v_sb")    # V natural [t-part, kb, feat]
            cc_sb = persist.tile([P, T], f32, tag="cc_sb")
            ss_sb = persist.tile([P, T], f32, tag="ss_sb")
            id_sb = persist.tile([P, P], bf16, tag="id_sb")
            ones_col = persist.tile([P, 1], bf16, tag="ones_col")    # sums lhsT
            ones_row = persist.tile([1, P], f32r, tag="ones_row")    # bcast lhsT
            eps_sb = persist.tile([P, 1], f32, tag="eps_sb")
            ones_f32 = persist.tile([P, 1], f32, tag="ones_f32")
            ones_row_f32 = persist.tile([1, P], f32, tag="ones_row_f32")
            nc.sync.dma_start(cc_sb[:], cc[:, :])
            nc.sync.dma_start(ss_sb[:], ss[:, :])
            nc.sync.dma_start(id_sb[:], ident[:, :])
            nc.vector.memset(eps_sb[:], EPS)
            nc.vector.memset(ones_f32[:], 1.0)
            nc.vector.memset(ones_row_f32[:], 1.0)
            nc.vector.tensor_copy(ones_col[:], ones_f32[:])
            nc.vector.tensor_copy(ones_row[:], ones_row_f32[:])

            # ---------------- Phase 1: QKV projections + RoPE + V transpose ----------------
            with (
                tc.tile_pool(name="ph1w", bufs=1) as wpool,
                tc.tile_pool(name="ph1x", bufs=2) as xpool,
                tc.tile_pool(name="ph1t", bufs=3) as tpool,
                tc.tile_pool(name="ph1ps", bufs=2, space="PSUM") as ps1,
            ):
                wq_sb = wpool.tile([P, KO, FQ], bf16, tag="wq_sb")
                wk_sb = wpool.tile([P, KO, FK], bf16, tag="wk_sb")
                wv_sb = wpool.tile([P, KO, FK], bf16, tag="wv_sb")
                nc.sync.dma_start(wq_sb[:], wq_r)
                nc.sync.dma_start(wk_sb[:], wk_r)
                nc.sync.dma_start(wv_sb[:], wv_r)

                for tch in range(NCHUNK):
                    t0 = tch * TCH
                    xt = xpool.tile([P, KO, TCH], bf16, tag="xt")
                    # per-ko DMAs so matmuls can start as slices land
                    for ko in range(KO):
                        nc.sync.dma_start(xt[:, ko, :], xT_r[:, ko, t0 : t0 + TCH])
                    # qT / kT feature blocks (4 q heads + 2 k heads)
                    for fb in range(NF):
                        if fb < NQ:
                            w_ap = wq_sb[:, :, fb * D : (fb + 1) * D]
                        else:
                            w_ap = wk_sb[:, :, (fb - NQ) * D : (fb - NQ + 1) * D]
                        pqk = ps1.tile([P, TCH], f32, tag="ps_qk")
                        for ko in range(KO):
                            nc.tensor.matmul(
                                pqk[:], w_ap[:, ko], xt[:, ko, :],
                                start=(ko == 0), stop=(ko == KO - 1),
                            )
                        # rope: raw chunk + half-swapped chunk (fp32), write bf16
                        raw = tpool.tile([P, TCH], f32, tag="rope_raw")
                        nc.vector.tensor_copy(raw[:], pqk[:])
                        swp = tpool.tile([P, TCH], f32, tag="rope_swp")
                        nc.sync.dma_start(swp[0:64, :], raw[64:128, :])
                        nc.sync.dma_start(swp[64:128, :], raw[0:64, :])
                        tmpa = tpool.tile([P, TCH], f32, tag="rope_tmpa")
                        tmpb = tpool.tile([P, TCH], f32, tag="rope_tmpb")
                        nc.vector.tensor_mul(tmpa[:], raw[:], cc_sb[:, t0 : t0 + TCH])
                        nc.vector.tensor_mul(tmpb[:], swp[:], ss_sb[:, t0 : t0 + TCH])
                        nc.vector.tensor_add(qk_rt[:, fb, t0 : t0 + TCH], tmpa[:], tmpb[:])
                    # vT blocks -> transpose -> V natural
                    for vfb in range(NK):
                        w_ap = wv_sb[:, :, vfb * D : (vfb + 1) * D]
                        pvt = ps1.tile([P, TCH], f32, tag="ps_vt")
                        for ko in range(KO):
                            nc.tensor.matmul(
                                pvt[:], w_ap[:, ko], xt[:, ko, :],
                                start=(ko == 0), stop=(ko == KO - 1),
                            )
                        vt_sb = tpool.tile([P, TCH], bf16, tag="vt_sb")
                        nc.vector.tensor_copy(vt_sb[:], pvt[:])
                        for tb in range(TCH // P):
                            ptr = ps1.tile([P, P], bf16, tag="ps_tr")
                            nc.tensor.transpose(
                                ptr[:], vt_sb[:, tb * P : (tb + 1) * P], id_sb[:]
                            )
                            nc.vector.tensor_copy(
                                v_sb[:, tch * (TCH // P) + tb, vfb * D : (vfb + 1) * D],
                                ptr[:],
                            )

            # ---------------- Phase 2: QK RMS norm (over head dim = partitions) ----------
            with (
                tc.tile_pool(name="ph2t", bufs=3) as p2,
                tc.tile_pool(name="ph2ps", bufs=2, space="PSUM") as ps2,
            ):
                for fb in range(NF):
                    for s in range(NSPAN):
                        c0 = s * SPAN
                        seg = qk_rt[:, fb, c0 : c0 + SPAN]
                        sq = p2.tile([P, SPAN], bf16, tag="sq")
                        nc.vector.tensor_mul(sq[:], seg, seg)
                        pms = ps2.tile([1, SPAN], f32, tag="ps_ms")
                        nc.tensor.matmul(pms[:], ones_col[:], sq[:], start=True, stop=True)
                        ms_sb = p2.tile([1, SPAN], f32r, tag="ms_sb")
                        nc.scalar.activation(ms_sb[:], pms[:], AF.Copy)
                        pb = ps2.tile([P, SPAN], f32, tag="ps_b")
                        nc.tensor.matmul(pb[:], ones_row[:], ms_sb[:], start=True, stop=True)
                        srt = p2.tile([P, SPAN], f32, tag="srt")
                        nc.scalar.activation(srt[:], pb[:], AF.Sqrt, bias=eps_sb[:], scale=1.0 / D)
                        rstd = p2.tile([P, SPAN], f32, tag="rstd")
                        nc.vector.reciprocal(rstd[:], srt[:])
                        nc.vector.tensor_mul(seg, seg, rstd[:])

            # ---------------- Phase 3: attention + Phase 4: output projection ------------
            with (
                tc.tile_pool(name="ph3s", bufs=1) as p3s,
                tc.tile_pool(name="ph3t", bufs=4) as p3,
            ):
                ot_sb = p3s.tile([P, NQ, T], bf16, tag="ot_sb")
                mask_sb = p3s.tile([P, 4, SPAN], f32, tag="mask_sb")
                wo_sb = p3s.tile([P, NQ, C], bf16, tag="wo_sb")
                nc.sync.dma_start(mask_sb[:], maskT[:, :, :])
                nc.sync.dma_start(wo_sb[:], wo_r)

                with (
                    tc.tile_pool(name="ph3ps", bufs=2, space="PSUM") as ps3,
                    tc.tile_pool(name="ph3ot", bufs=1, space="PSUM") as psot,
                ):
                  for h in range(NQ):
                    j = h // 2
                    for s in range(NSPAN):
                        q0 = s * SPAN
                        nkb = 4 * s + 4
                        ot_ps = psot.tile([P, SPAN], f32, tag="ot_ps")
                        sum_ps = psot.tile([1, SPAN], f32, tag="sum_ps")
                        q_ap = qk_rt[:, h, q0 : q0 + SPAN]
                        for kb in range(nkb):
                            st_ps = ps3.tile([P, SPAN], f32, tag="st_ps")
                            nc.tensor.matmul(
                                st_ps[:],
                                qk_rt[:, NQ + j, kb * P : (kb + 1) * P],
                                q_ap,
                                start=True, stop=True,
                            )
                            pt = p3.tile([P, SPAN], bf16, tag="pt")
                            nc.scalar.activation(pt[:], st_ps[:], AF.Exp, scale=SCALE)
                            if kb >= 4 * s:
                                nc.vector.tensor_mul(
                                    pt[:], pt[:], mask_sb[:, kb - 4 * s, :]
                                )
                            nc.tensor.matmul(
                                ot_ps[:],
                                v_sb[:, kb, j * D : (j + 1) * D],
                                pt[:],
                                start=(kb == 0), stop=(kb == nkb - 1),
                                skip_group_check=True,
                            )
                            nc.tensor.matmul(
                                sum_ps[:],
                                ones_col[:],
                                pt[:],
                                start=(kb == 0), stop=(kb == nkb - 1),
                                skip_group_check=True,
                            )
                        # normalization (fp32 chain): bcast sums, reciprocal on 128 lanes
                        sum_sb = p3.tile([1, SPAN], f32r, tag="sum_sb")
                        nc.scalar.activation(sum_sb[:], sum_ps[:], AF.Copy)
                        bc_ps = ps3.tile([P, SPAN], f32, tag="bc_ps")
                        nc.tensor.matmul(bc_ps[:], ones_row[:], sum_sb[:], start=True, stop=True)
                        bc_sb = p3.tile([P, SPAN], f32, tag="bc_sb")
                        nc.vector.reciprocal(bc_sb[:], bc_ps[:])
                        nc.vector.tensor_mul(
                            ot_sb[:, h, q0 : q0 + SPAN], ot_ps[:], bc_sb[:]
                        )

                # output projection: y[tb, n] += OT.T @ wo_rows
                with (
                    tc.tile_pool(name="ph4t", bufs=3) as p4,
                    tc.tile_pool(name="ph4ps", bufs=3, space="PSUM") as ps4,
                ):
                    for tb in range(T // P):
                        for nch in range(C // 512):
                            yps = ps4.tile([P, 512], f32, tag="yps")
                            for h in range(NQ):
                                nc.tensor.matmul(
                                    yps[:],
                                    ot_sb[:, h, tb * P : (tb + 1) * P],
                                    wo_sb[:, h, nch * 512 : (nch + 1) * 512],
                                    start=(h == 0), stop=(h == NQ - 1),
                                )
                            ysb = p4.tile([P, 512], f32, tag="ysb")
                            nc.vector.tensor_copy(ysb[:], yps[:])
                            nc.sync.dma_start(
                                y[tb * P : (tb + 1) * P, nch * 512 : (nch + 1) * 512],
                                ysb[:],
                            )
    nc.compile()
    return nc


_NC_CACHE = None


def _get_nc():
    global _NC_CACHE
    if _NC_CACHE is None:
        _NC_CACHE = build()
    return _NC_CACHE


def _host_inputs(x, cos, sin, wq, wk, wv, wo):
    """Build the 8 per-core input maps."""
    bft = ml_dtypes.bfloat16
    cosT = np.ascontiguousarray(cos[0, :, 0, :].T).astype(np.float32)  # (64, T)
    sinT = np.ascontiguousarray(sin[0, :, 0, :].T).astype(np.float32)
    cc = np.concatenate([cosT, cosT], axis=0)          # (128, T)
    ss = np.concatenate([sinT, -sinT], axis=0)
    # maskT[r][k, q] = 1 if q >= 128*r + k  (within a 512-q span, k-block offset r)
    qidx = np.arange(SPAN)[None, None, :]
    kidx = np.arange(P)[:, None, None]
    ridx = np.arange(4)[None, :, None]
    maskT = (qidx >= P * ridx + kidx).astype(np.float32)  # (128, 4, 512)
    ident = np.eye(P, dtype=np.float32).astype(bft)

    xTs = [np.ascontiguousarray(x[b].T).astype(bft) for b in range(2)]
    wq16 = wq.astype(bft)
    wk16 = wk.astype(bft)
    wv16 = wv.astype(bft)
    wo16 = wo.astype(bft)
    in_maps = []
    for c in range(8):
        b, tp = divmod(c, 4)
        in_maps.append(
            {
                "xT": xTs[b],
                "wq": np.ascontiguousarray(wq16[:, tp * FQ : (tp + 1) * FQ]),
                "wk": np.ascontiguousarray(wk16[:, tp * FK : (tp + 1) * FK]),
                "wv": np.ascontiguousarray(wv16[:, tp * FK : (tp + 1) * FK]),
                "wo": np.ascontiguousarray(wo16[tp * FQ : (tp + 1) * FQ, :]),
                "cc": cc,
                "ss": ss,
                "maskT": maskT,
                "ident": ident,
            }
        )
    return in_maps


def kernel(x, cos, sin, wq, wk, wv, wo, trace=False):
    x = np.asarray(x, dtype=np.float32)
    cos = np.asarray(cos, dtype=np.float32)
    sin = np.asarray(sin, dtype=np.float32)
    wq = np.asarray(wq, dtype=np.float32)
    wk = np.asarray(wk, dtype=np.float32)
    wv = np.asarray(wv, dtype=np.float32)
    wo = np.asarray(wo, dtype=np.float32)

    nc = _get_nc()
    in_maps = _host_inputs(x, cos, sin, wq, wk, wv, wo)
    res = run_bass_kernel_spmd(nc, in_maps, core_ids=list(range(8)), trace=trace)
    out = np.zeros((2, T, C), dtype=np.float32)
    for c in range(8):
        b = c // 4
        out[b] += res.results[c]["y"]
    if trace:
        return out, res
    return out


# revision 10
# speedup vs baseline: 1.1850x; 1.1199x over previous
"""Causal self-attention (RoPE + QK-RMSNorm, GQA 16q/8kv) Trainium2 Bass kernel.

Sharding: 8 cores = 2 batch x 4 tensor-parallel. Core c handles batch b=c//4 and
q-heads [4*tp, 4*tp+4), kv-heads [2*tp, 2*tp+2) where tp=c%4. Each core returns a
partial (T, C) output = O_heads @ wo[rows of its heads]; host sums the 4 partials
per batch (the "all-reduce after c_proj").

Matmuls run in bf16 (fp32 PSUM accumulation); softmax row-sum normalization and
RMS statistics stay in fp32/fp32r.
"""
import sys
import math

sys.path.insert(0, "/opt/trn_rl_repo")

import numpy as np
import ml_dtypes
import concourse.bacc as bacc
import concourse.mybir as mybir
import concourse.tile as tile
from concourse.bass_utils import run_bass_kernel_spmd

P = 128
T = 2048
C = 2048
KO = C // P          # 16 contraction tiles
D = 128              # head dim
NQ = 4               # q heads per core
NK = 2               # kv heads per core
NF = NQ + NK         # 6 rope/rms feature blocks (4 q + 2 k)
FQ = NQ * D          # 512
FK = NK * D          # 256
TCH = 512            # phase-1 T-chunk
NCHUNK = T // TCH    # 4
SPAN = 512           # attention q-span
NSPAN = T // SPAN    # 4
KB = T // P          # 16 key blocks
SCALE = 1.0 / math.sqrt(D)
EPS = 1.1920929e-07

f32 = mybir.dt.float32
f32r = mybir.dt.float32r
bf16 = mybir.dt.bfloat16

AF = mybir.ActivationFunctionType


def build():
    nc = bacc.Bacc("TRN2", target_bir_lowering=False)
    xT = nc.dram_tensor("xT", (C, T), bf16, kind="ExternalInput")
    wq = nc.dram_tensor("wq", (C, FQ), bf16, kind="ExternalInput")
    wk = nc.dram_tensor("wk", (C, FK), bf16, kind="ExternalInput")
    wv = nc.dram_tensor("wv", (C, FK), bf16, kind="ExternalInput")
    wo = nc.dram_tensor("wo", (FQ, C), bf16, kind="ExternalInput")
    cc = nc.dram_tensor("cc", (P, T), f32, kind="ExternalInput")    # [cos; cos]
    ss = nc.dram_tensor("ss", (P, T), f32, kind="ExternalInput")    # [sin; -sin]
    maskT = nc.dram_tensor("maskT", (P, 4, SPAN), f32, kind="ExternalInput")
    ident = nc.dram_tensor("ident", (P, P), bf16, kind="ExternalInput")
    y = nc.dram_tensor("y", (T, C), f32, kind="ExternalOutput")

    xT_r = xT.rearrange("(ko p) t -> p ko t", p=P)
    wq_r = wq.rearrange("(ko p) f -> p ko f", p=P)
    wk_r = wk.rearrange("(ko p) f -> p ko f", p=P)
    wv_r = wv.rearrange("(ko p) f -> p ko f", p=P)
    wo_r = wo.rearrange("(ko p) n -> p ko n", p=P)

    with tile.TileContext(nc) as tc:
        with tc.tile_pool(name="persist", bufs=1) as persist:
            # persistent across phases
            qk_rt = persist.tile([P, NF, T], bf16, tag="qk_rt")   # roped+normed qT/kT
            v_sb = persist.tile([P, KB, FK], bf16, tag="v_sb")    # V natural [t-part, kb, feat]
            cc_sb = persist.tile([P, T], f32, tag="cc_sb")
            ss_sb = persist.tile([P, T], f32, tag="ss_sb")
            id_sb = persist.tile([P, P], bf16, tag="id_sb")
            ones_col = persist.tile([P, 1], bf16, tag="ones_col")    # sums lhsT
            ones_row = persist.tile([1, P], f32r, tag="ones_row")    # bcast lhsT
            eps_sb = persist.tile([P, 1], f32, tag="eps_sb")
            ones_f32 = persist.tile([P, 1], f32, tag="ones_f32")
            ones_row_f32 = persist.tile([1, P], f32, tag="ones_row_f32")
            nc.sync.dma_start(cc_sb[:], cc[:, :])
            nc.sync.dma_start(ss_sb[:], ss[:, :])
            nc.sync.dma_start(id_sb[:], ident[:, :])
            nc.vector.memset(eps_sb[:], EPS)
            nc.vector.memset(ones_f32[:], 1.0)
            nc.vector.memset(ones_row_f32[:], 1.0)
            nc.vector.tensor_copy(ones_col[:], ones_f32[:])
            nc.vector.tensor_copy(ones_row[:], ones_row_f32[:])

            # ------- Phase 1: QKV projections + RoPE + RMS norm + V transpose -------
            with (
                tc.tile_pool(name="ph1w", bufs=1) as wpool,
                tc.tile_pool(name="ph1x", bufs=2) as xpool,
                tc.tile_pool(name="ph1t", bufs=3) as tpool,
                tc.tile_pool(name="ph1ps", bufs=3, space="PSUM") as ps1,
                tc.tile_pool(name="ph1tr", bufs=1, space="PSUM") as pstr,
                tc.tile_pool(name="ph1ms", bufs=2, space="PSUM") as psms,
                tc.tile_pool(name="ph1rb", bufs=2, space="PSUM") as psrb,
            ):
                wq_sb = wpool.tile([P, KO, FQ], bf16, tag="wq_sb")
                wk_sb = wpool.tile([P, KO, FK], bf16, tag="wk_sb")
                wv_sb = wpool.tile([P, KO, FK], bf16, tag="wv_sb")
                nc.sync.dma_start(wq_sb[:], wq_r)
                nc.sync.dma_start(wk_sb[:], wk_r)
                nc.sync.dma_start(wv_sb[:], wv_r)

                for tch in range(NCHUNK):
                    t0 = tch * TCH
                    xt = xpool.tile([P, KO, TCH], bf16, tag="xt")
                    # per-ko DMAs so matmuls can start as slices land
                    for ko in range(KO):
                        nc.sync.dma_start(xt[:, ko, :], xT_r[:, ko, t0 : t0 + TCH])
                    # qT / kT feature blocks (4 q heads + 2 k heads)
                    for fb in range(NF):
                        if fb < NQ:
                            w_ap = wq_sb[:, :, fb * D : (fb + 1) * D]
                        else:
                            w_ap = wk_sb[:, :, (fb - NQ) * D : (fb - NQ + 1) * D]
                        pqk = ps1.tile([P, TCH], f32, tag="ps_qkv")
                        for ko in range(KO):
                            nc.tensor.matmul(
                                pqk[:], w_ap[:, ko], xt[:, ko, :],
                                start=(ko == 0), stop=(ko == KO - 1),
                            )
                        # rope: raw chunk + half-swapped chunk (fp32), write bf16
                        raw = tpool.tile([P, TCH], f32, tag="rope_raw")
                        nc.vector.tensor_copy(raw[:], pqk[:])
                        swp = tpool.tile([P, TCH], f32, tag="rope_swp")
                        nc.sync.dma_start(swp[0:64, :], raw[64:128, :])
                        nc.sync.dma_start(swp[64:128, :], raw[0:64, :])
                        tmpa = tpool.tile([P, TCH], f32, tag="rope_tmpa")
                        tmpb = tpool.tile([P, TCH], f32, tag="rope_tmpb")
                        seg = qk_rt[:, fb, t0 : t0 + TCH]
                        nc.vector.tensor_mul(tmpa[:], raw[:], cc_sb[:, t0 : t0 + TCH])
                        nc.vector.tensor_mul(tmpb[:], swp[:], ss_sb[:, t0 : t0 + TCH])
                        nc.vector.tensor_add(seg, tmpa[:], tmpb[:])
                        # RMS over head dim (partitions): ones-matmul + rank-1 bcast
                        sq = tpool.tile([P, TCH], bf16, tag="sq")
                        nc.vector.tensor_mul(sq[:], seg, seg)
                        pms = psms.tile([1, TCH], f32, tag="ps_ms")
                        nc.tensor.matmul(pms[:], ones_col[:], sq[:], start=True, stop=True)
                        srt = tpool.tile([1, TCH], f32, tag="srt")
                        nc.scalar.activation(
                            srt[:], pms[:], AF.Sqrt, bias=eps_sb[0:1, :], scale=1.0 / D
                        )
                        rstd = tpool.tile([1, TCH], f32r, tag="rstd")
                        with nc.allow_low_precision(reason="fp32r rounding for bcast matmul"):
                            nc.vector.reciprocal(rstd[:], srt[:])
                        pb = psrb.tile([P, TCH], f32, tag="ps_b")
                        nc.tensor.matmul(pb[:], ones_row[:], rstd[:], start=True, stop=True)
                        nc.vector.tensor_mul(seg, seg, pb[:])
                    # vT blocks -> transpose -> V natural
                    for vfb in range(NK):
                        w_ap = wv_sb[:, :, vfb * D : (vfb + 1) * D]
                        pvt = ps1.tile([P, TCH], f32, tag="ps_qkv")
                        for ko in range(KO):
                            nc.tensor.matmul(
                                pvt[:], w_ap[:, ko], xt[:, ko, :],
                                start=(ko == 0), stop=(ko == KO - 1),
                            )
                        vt_sb = tpool.tile([P, TCH], bf16, tag="vt_sb")
                        nc.vector.tensor_copy(vt_sb[:], pvt[:])
                        for tb in range(TCH // P):
                            ptr = pstr.tile([P, P], bf16, tag="ps_tr")
                            nc.tensor.transpose(
                                ptr[:], vt_sb[:, tb * P : (tb + 1) * P], id_sb[:]
                            )
                            nc.vector.tensor_copy(
                                v_sb[:, tch * (TCH // P) + tb, vfb * D : (vfb + 1) * D],
                                ptr[:],
                            )

            # ---------------- Phase 3: attention + Phase 4: output projection ------------
            with (
                tc.tile_pool(name="ph3s", bufs=1) as p3s,
                tc.tile_pool(name="ph3t", bufs=4) as p3,
            ):
                ot_sb = p3s.tile([P, NQ, T], bf16, tag="ot_sb")
                mask_sb = p3s.tile([P, 4, SPAN], f32, tag="mask_sb")
                wo_sb = p3s.tile([P, NQ, C], bf16, tag="wo_sb")
                nc.sync.dma_start(mask_sb[:], maskT[:, :, :])
                nc.sync.dma_start(wo_sb[:], wo_r)

                with (
                    tc.tile_pool(name="ph3ps", bufs=2, space="PSUM") as ps3,
                    tc.tile_pool(name="ph3ot", bufs=2, space="PSUM") as psot,
                    tc.tile_pool(name="ph3m", bufs=2, space="PSUM") as psm,
                ):
                  for s in range(NSPAN):
                    q0 = s * SPAN
                    nkb = 4 * s + 4
                    for h in range(NQ):
                        j = h // 2
                        ot_ps = psot.tile([P, SPAN], f32, tag="ot_ps")
                        sum_ps = psot.tile([1, SPAN], f32, tag="sum_ps")
                        q_ap = qk_rt[:, h, q0 : q0 + SPAN]
                        for kb in range(nkb):
                            st_ps = ps3.tile([P, SPAN], f32, tag="st_ps")
                            nc.tensor.matmul(
                                st_ps[:],
                                qk_rt[:, NQ + j, kb * P : (kb + 1) * P],
                                q_ap,
                                start=True, stop=True,
                            )
                            pt = p3.tile([P, SPAN], bf16, tag="pt")
                            nc.scalar.activation(pt[:], st_ps[:], AF.Exp, scale=SCALE)
                            if kb >= 4 * s:
                                nc.vector.tensor_mul(
                                    pt[:], pt[:], mask_sb[:, kb - 4 * s, :]
                                )
                            nc.tensor.matmul(
                                ot_ps[:],
                                v_sb[:, kb, j * D : (j + 1) * D],
                                pt[:],
                                start=(kb == 0), stop=(kb == nkb - 1),
                                skip_group_check=True,
                            )
                            nc.tensor.matmul(
                                sum_ps[:],
                                ones_col[:],
                                pt[:],
                                start=(kb == 0), stop=(kb == nkb - 1),
                                skip_group_check=True,
                            )
                        # normalization: 1/sums then rank-1 bcast matmul (fp32r)
                        rec = p3.tile([1, SPAN], f32r, tag="rec")
                        with nc.allow_low_precision(reason="fp32r rounding for bcast matmul"):
                            nc.vector.reciprocal(rec[:], sum_ps[:])
                        bc_ps = psm.tile([P, SPAN], f32, tag="m512")
                        nc.tensor.matmul(bc_ps[:], ones_row[:], rec[:], start=True, stop=True)
                        bc_sb = p3.tile([P, SPAN], f32, tag="bc_sb")
                        nc.scalar.activation(bc_sb[:], bc_ps[:], AF.Copy)
                        nc.vector.tensor_mul(
                            ot_sb[:, h, q0 : q0 + SPAN], ot_ps[:], bc_sb[:]
                        )

                    # output projection for the T-blocks of this span
                    for tb in range(4 * s, 4 * s + 4):
                        for nch in range(C // 512):
                            yps = psm.tile([P, 512], f32, tag="m512")
                            for h in range(NQ):
                                nc.tensor.matmul(
                                    yps[:],
                                    ot_sb[:, h, tb * P : (tb + 1) * P],
                                    wo_sb[:, h, nch * 512 : (nch + 1) * 512],
                                    start=(h == 0), stop=(h == NQ - 1),
                                )
                            ysb = p3.tile([P, 512], f32, tag="ysb")
                            nc.vector.tensor_copy(ysb[:], yps[:])
                            nc.sync.dma_start(
                                y[tb * P : (tb + 1) * P, nch * 512 : (nch + 1) * 512],
                                ysb[:],
                            )
    nc.compile()
    return nc


_NC_CACHE = None


def _get_nc():
    global _NC_CACHE
    if _NC_CACHE is None:
        _NC_CACHE = build()
    return _NC_CACHE


def _host_inputs(x, cos, sin, wq, wk, wv, wo):
    """Build the 8 per-core input maps."""
    bft = ml_dtypes.bfloat16
    cosT = np.ascontiguousarray(cos[0, :, 0, :].T).astype(np.float32)  # (64, T)
    sinT = np.ascontiguousarray(sin[0, :, 0, :].T).astype(np.float32)
    cc = np.concatenate([cosT, cosT], axis=0)          # (128, T)
    ss = np.concatenate([sinT, -sinT], axis=0)
    # maskT[r][k, q] = 1 if q >= 128*r + k  (within a 512-q span, k-block offset r)
    qidx = np.arange(SPAN)[None, None, :]
    kidx = np.arange(P)[:, None, None]
    ridx = np.arange(4)[None, :, None]
    maskT = (qidx >= P * ridx + kidx).astype(np.float32)  # (128, 4, 512)
    ident = np.eye(P, dtype=np.float32).astype(bft)

    xTs = [np.ascontiguousarray(x[b].T).astype(bft) for b in range(2)]
    wq16 = wq.astype(bft)
    wk16 = wk.astype(bft)
    wv16 = wv.astype(bft)
    wo16 = wo.astype(bft)
    in_maps = []
    for c in range(8):
        b, tp = divmod(c, 4)
        in_maps.append(
            {
                "xT": xTs[b],
                "wq": np.ascontiguousarray(wq16[:, tp * FQ : (tp + 1) * FQ]),
                "wk": np.ascontiguousarray(wk16[:, tp * FK : (tp + 1) * FK]),
                "wv": np.ascontiguousarray(wv16[:, tp * FK : (tp + 1) * FK]),
                "wo": np.ascontiguousarray(wo16[tp * FQ : (tp + 1) * FQ, :]),
                "cc": cc,
                "ss": ss,
                "maskT": maskT,
                "ident": ident,
            }
        )
    return in_maps


def kernel(x, cos, sin, wq, wk, wv, wo, trace=False):
    x = np.asarray(x, dtype=np.float32)
    cos = np.asarray(cos, dtype=np.float32)
    sin = np.asarray(sin, dtype=np.float32)
    wq = np.asarray(wq, dtype=np.float32)
    wk = np.asarray(wk, dtype=np.float32)
    wv = np.asarray(wv, dtype=np.float32)
    wo = np.asarray(wo, dtype=np.float32)

    nc = _get_nc()
    in_maps = _host_inputs(x, cos, sin, wq, wk, wv, wo)
    res = run_bass_kernel_spmd(nc, in_maps, core_ids=list(range(8)), trace=trace)
    out = np.zeros((2, T, C), dtype=np.float32)
    for c in range(8):
        b = c // 4
        out[b] += res.results[c]["y"]
    if trace:
        return out, res
    return out


# revision 14
# speedup vs baseline: 1.3246x; 1.1178x over previous
"""Causal self-attention (RoPE + QK-RMSNorm, GQA 16q/8kv) Trainium2 Bass kernel.

Sharding: 8 cores = 2 batch x 4 tensor-parallel. Core c handles batch b=c//4 and
q-heads [4*tp, 4*tp+4), kv-heads [2*tp, 2*tp+2) where tp=c%4. Each core returns a
partial (T, C) output = O_heads @ wo[rows of its heads]; host sums the 4 partials
per batch (the "all-reduce after c_proj").

Matmuls run in bf16 (fp32 PSUM accumulation); softmax row-sum normalization and
RMS statistics stay in fp32/fp32r.
"""
import sys
import math

sys.path.insert(0, "/opt/trn_rl_repo")

import numpy as np
import ml_dtypes
import concourse.bacc as bacc
import concourse.mybir as mybir
import concourse.tile as tile
from concourse.bass_utils import run_bass_kernel_spmd

P = 128
T = 2048
C = 2048
KO = C // P          # 16 contraction tiles
D = 128              # head dim
NQ = 4               # q heads per core
NK = 2               # kv heads per core
NF = NQ + NK         # 6 rope/rms feature blocks (4 q + 2 k)
FQ = NQ * D          # 512
FK = NK * D          # 256
TCH = 512            # phase-1 T-chunk
NCHUNK = T // TCH    # 4
SPAN = 512           # attention q-span
NSPAN = T // SPAN    # 4
KB = T // P          # 16 key blocks
SCALE = 1.0 / math.sqrt(D)
EPS = 1.1920929e-07

f32 = mybir.dt.float32
f32r = mybir.dt.float32r
bf16 = mybir.dt.bfloat16

AF = mybir.ActivationFunctionType


def build():
    nc = bacc.Bacc("TRN2", target_bir_lowering=False)
    xT = nc.dram_tensor("xT", (C, T), bf16, kind="ExternalInput")
    wq = nc.dram_tensor("wq", (C, FQ), bf16, kind="ExternalInput")
    wk = nc.dram_tensor("wk", (C, FK), bf16, kind="ExternalInput")
    wv = nc.dram_tensor("wv", (C, FK), bf16, kind="ExternalInput")
    wo = nc.dram_tensor("wo", (FQ, C), bf16, kind="ExternalInput")
    cc = nc.dram_tensor("cc", (P, T), f32, kind="ExternalInput")    # [cos; cos]
    ss = nc.dram_tensor("ss", (P, T), f32, kind="ExternalInput")    # [sin; -sin]
    maskT = nc.dram_tensor("maskT", (P, 4, SPAN), bf16, kind="ExternalInput")
    ident = nc.dram_tensor("ident", (P, P), bf16, kind="ExternalInput")
    y = nc.dram_tensor("y", (T, C), f32, kind="ExternalOutput")

    xT_r = xT.rearrange("(ko p) t -> p ko t", p=P)
    wq_r = wq.rearrange("(ko p) f -> p ko f", p=P)
    wk_r = wk.rearrange("(ko p) f -> p ko f", p=P)
    wv_r = wv.rearrange("(ko p) f -> p ko f", p=P)
    wo_r = wo.rearrange("(ko p) n -> p ko n", p=P)

    with tile.TileContext(nc) as tc:
        with tc.tile_pool(name="persist", bufs=1) as persist:
            # persistent across phases
            qk_rt = persist.tile([P, NF, T], bf16, tag="qk_rt")   # roped+normed qT/kT
            v_sb = persist.tile([P, KB, FK], bf16, tag="v_sb")    # V natural [t-part, kb, feat]
            cc_sb = persist.tile([P, T], f32, tag="cc_sb")
            ss_sb = persist.tile([P, T], f32, tag="ss_sb")
            id_sb = persist.tile([P, P], bf16, tag="id_sb")
            ones_col = persist.tile([P, 1], bf16, tag="ones_col")    # sums lhsT
            ones_row = persist.tile([1, P], f32r, tag="ones_row")    # bcast lhsT
            eps_sb = persist.tile([P, 1], f32, tag="eps_sb")
            zero_sb = persist.tile([1, 1], f32, tag="zero_sb")
            nc.vector.memset(zero_sb[:], 0.0)
            ones_f32 = persist.tile([P, 1], f32, tag="ones_f32")
            ones_row_f32 = persist.tile([1, P], f32, tag="ones_row_f32")
            nc.sync.dma_start(cc_sb[:], cc[:, :])
            nc.sync.dma_start(ss_sb[:], ss[:, :])
            nc.sync.dma_start(id_sb[:], ident[:, :])
            nc.vector.memset(eps_sb[:], EPS)
            nc.vector.memset(ones_f32[:], 1.0)
            nc.vector.memset(ones_row_f32[:], 1.0)
            nc.vector.tensor_copy(ones_col[:], ones_f32[:])
            nc.vector.tensor_copy(ones_row[:], ones_row_f32[:])

            # ------- Phase 1: QKV projections + RoPE + RMS norm + V transpose -------
            with (
                tc.tile_pool(name="ph1w", bufs=1) as wpool,
                tc.tile_pool(name="ph1x", bufs=2) as xpool,
                tc.tile_pool(name="ph1t", bufs=3) as tpool,
                tc.tile_pool(name="ph1ps", bufs=3, space="PSUM") as ps1,
                tc.tile_pool(name="ph1tr", bufs=1, space="PSUM") as pstr,
                tc.tile_pool(name="ph1ms", bufs=2, space="PSUM") as psms,
                tc.tile_pool(name="ph1rb", bufs=2, space="PSUM") as psrb,
            ):
                wq_sb = wpool.tile([P, KO, FQ], bf16, tag="wq_sb")
                wk_sb = wpool.tile([P, KO, FK], bf16, tag="wk_sb")
                wv_sb = wpool.tile([P, KO, FK], bf16, tag="wv_sb")
                nc.sync.dma_start(wq_sb[:], wq_r)
                nc.sync.dma_start(wk_sb[:], wk_r)
                nc.sync.dma_start(wv_sb[:], wv_r)

                for tch in range(NCHUNK):
                    t0 = tch * TCH
                    xt = xpool.tile([P, KO, TCH], bf16, tag="xt")
                    # per-ko DMAs so matmuls can start as slices land
                    for ko in range(KO):
                        nc.sync.dma_start(xt[:, ko, :], xT_r[:, ko, t0 : t0 + TCH])
                    # qT / kT feature blocks (4 q heads + 2 k heads)
                    sqs = []
                    for fb in range(NF):
                        if fb < NQ:
                            w_ap = wq_sb[:, :, fb * D : (fb + 1) * D]
                        else:
                            w_ap = wk_sb[:, :, (fb - NQ) * D : (fb - NQ + 1) * D]
                        pqk = ps1.tile([P, TCH], f32, tag="ps_qkv")
                        for ko in range(KO):
                            nc.tensor.matmul(
                                pqk[:], w_ap[:, ko], xt[:, ko, :],
                                start=(ko == 0), stop=(ko == KO - 1),
                            )
                        # rope: raw chunk + half-swapped chunk (fp32), write bf16
                        raw = tpool.tile([P, TCH], f32, tag="rope_raw")
                        nc.vector.tensor_copy(raw[:], pqk[:])
                        swp = tpool.tile([P, TCH], f32, tag="rope_swp")
                        nc.sync.dma_start(swp[0:64, :], raw[64:128, :])
                        nc.sync.dma_start(swp[64:128, :], raw[0:64, :])
                        tmpa = tpool.tile([P, TCH], f32, tag="rope_tmpa")
                        tmpb = tpool.tile([P, TCH], f32, tag="rope_tmpb")
                        seg = qk_rt[:, fb, t0 : t0 + TCH]
                        nc.vector.tensor_mul(tmpa[:], raw[:], cc_sb[:, t0 : t0 + TCH])
                        nc.vector.tensor_mul(tmpb[:], swp[:], ss_sb[:, t0 : t0 + TCH])
                        nc.vector.tensor_add(seg, tmpa[:], tmpb[:])
                        # RMS stats: sum of squares over head dim (partitions)
                        sq = tpool.tile([P, TCH], bf16, tag="sq")
                        nc.vector.tensor_mul(sq[:], seg, seg)
                        pms = psms.tile([1, TCH], f32, tag="ps_ms")
                        nc.tensor.matmul(pms[:], ones_col[:], sq[:], start=True, stop=True)
                        # rstd = exp(-0.5 * ln(ms/D + eps)) — both on ACT, off the PE path
                        lnms = tpool.tile([1, TCH], f32, tag="lnms")
                        nc.scalar.activation(
                            lnms[:], pms[:], AF.Ln, bias=eps_sb[0:1, :], scale=1.0 / D
                        )
                        rstd = tpool.tile([1, TCH], f32r, tag="rstd")
                        nc.scalar.activation(rstd[:], lnms[:], AF.Exp, scale=-0.5)
                        sqs.append((seg, rstd))
                    # RMS apply pass — bcast matmuls run a full block later so the
                    # ACT chain has drained and the PE never head-of-line blocks
                    for seg, rstd in sqs:
                        pb = psrb.tile([P, TCH], f32, tag="ps_b")
                        nc.tensor.matmul(pb[:], ones_row[:], rstd[:], start=True, stop=True)
                        nc.vector.tensor_mul(seg, seg, pb[:])
                    # vT blocks -> transpose -> V natural
                    for vfb in range(NK):
                        w_ap = wv_sb[:, :, vfb * D : (vfb + 1) * D]
                        pvt = ps1.tile([P, TCH], f32, tag="ps_qkv")
                        for ko in range(KO):
                            nc.tensor.matmul(
                                pvt[:], w_ap[:, ko], xt[:, ko, :],
                                start=(ko == 0), stop=(ko == KO - 1),
                            )
                        vt_sb = tpool.tile([P, TCH], bf16, tag="vt_sb")
                        nc.vector.tensor_copy(vt_sb[:], pvt[:])
                        for tb in range(TCH // P):
                            ptr = pstr.tile([P, P], bf16, tag="ps_tr")
                            nc.tensor.transpose(
                                ptr[:], vt_sb[:, tb * P : (tb + 1) * P], id_sb[:]
                            )
                            nc.vector.tensor_copy(
                                v_sb[:, tch * (TCH // P) + tb, vfb * D : (vfb + 1) * D],
                                ptr[:],
                            )

            # ---------------- Phase 3: attention + Phase 4: output projection ------------
            with (
                tc.tile_pool(name="ph3s", bufs=1) as p3s,
                tc.tile_pool(name="ph3t", bufs=4) as p3,
            ):
                ot_sb = p3s.tile([P, NQ, T], bf16, tag="ot_sb")
                mask_sb = p3s.tile([P, 4, SPAN], bf16, tag="mask_sb")
                wo_sb = p3s.tile([P, NQ, C], bf16, tag="wo_sb")
                nc.sync.dma_start(mask_sb[:], maskT[:, :, :])
                nc.sync.dma_start(wo_sb[:], wo_r)

                with (
                    tc.tile_pool(name="ph3ps", bufs=2, space="PSUM") as ps3,
                    tc.tile_pool(name="ph3ot", bufs=2, space="PSUM") as psot,
                    tc.tile_pool(name="ph3m", bufs=2, space="PSUM") as psm,
                ):
                  for s in range(NSPAN):
                    q0 = s * SPAN
                    nkb = 4 * s + 4
                    for h in range(NQ):
                        j = h // 2
                        ot_ps = psot.tile([P, SPAN], f32, tag="ot_ps")
                        sum_ps = psot.tile([1, SPAN], f32, tag="sum_ps")
                        q_ap = qk_rt[:, h, q0 : q0 + SPAN]
                        for kb in range(nkb):
                            st_ps = ps3.tile([P, SPAN], f32, tag="st_ps")
                            nc.tensor.matmul(
                                st_ps[:],
                                qk_rt[:, NQ + j, kb * P : (kb + 1) * P],
                                q_ap,
                                start=True, stop=True,
                            )
                            pt = p3.tile([P, SPAN], bf16, tag="pt")
                            nc.scalar.activation(pt[:], st_ps[:], AF.Exp, scale=SCALE)
                            if kb >= 4 * s:
                                nc.vector.tensor_mul(
                                    pt[:], pt[:], mask_sb[:, kb - 4 * s, :]
                                )
                            nc.tensor.matmul(
                                ot_ps[:],
                                v_sb[:, kb, j * D : (j + 1) * D],
                                pt[:],
                                start=(kb == 0), stop=(kb == nkb - 1),
                                skip_group_check=True,
                            )
                            nc.tensor.matmul(
                                sum_ps[:],
                                ones_col[:],
                                pt[:],
                                start=(kb == 0), stop=(kb == nkb - 1),
                                skip_group_check=True,
                            )
                        # normalization: 1/sums = exp(-ln(sums)) on ACT, then bcast
                        lns = p3.tile([1, SPAN], f32, tag="lns")
                        nc.scalar.activation(lns[:], sum_ps[:], AF.Ln)
                        rec = p3.tile([1, SPAN], f32r, tag="rec")
                        nc.scalar.activation(rec[:], lns[:], AF.Exp, scale=-1.0)
                        bc_ps = psm.tile([P, SPAN], f32, tag="m512")
                        nc.tensor.matmul(bc_ps[:], ones_row[:], rec[:], start=True, stop=True)
                        bc_sb = p3.tile([P, SPAN], f32, tag="bc_sb")
                        nc.scalar.activation(bc_sb[:], bc_ps[:], AF.Copy)
                        nc.vector.tensor_mul(
                            ot_sb[:, h, q0 : q0 + SPAN], ot_ps[:], bc_sb[:]
                        )

                    # output projection for the T-blocks of this span
                    for tb in range(4 * s, 4 * s + 4):
                        for nch in range(C // 512):
                            yps = psm.tile([P, 512], f32, tag="m512")
                            for h in range(NQ):
                                nc.tensor.matmul(
                                    yps[:],
                                    ot_sb[:, h, tb * P : (tb + 1) * P],
                                    wo_sb[:, h, nch * 512 : (nch + 1) * 512],
                                    start=(h == 0), stop=(h == NQ - 1),
                                )
                            ysb = p3.tile([P, 512], f32, tag="ysb")
                            nc.vector.tensor_copy(ysb[:], yps[:])
                            nc.sync.dma_start(
                                y[tb * P : (tb + 1) * P, nch * 512 : (nch + 1) * 512],
                                ysb[:],
                            )
    nc.compile()
    return nc


_NC_CACHE = None


def _get_nc():
    global _NC_CACHE
    if _NC_CACHE is None:
        _NC_CACHE = build()
    return _NC_CACHE


def _host_inputs(x, cos, sin, wq, wk, wv, wo):
    """Build the 8 per-core input maps."""
    bft = ml_dtypes.bfloat16
    cosT = np.ascontiguousarray(cos[0, :, 0, :].T).astype(np.float32)  # (64, T)
    sinT = np.ascontiguousarray(sin[0, :, 0, :].T).astype(np.float32)
    cc = np.concatenate([cosT, cosT], axis=0)          # (128, T)
    ss = np.concatenate([sinT, -sinT], axis=0)
    # maskT[r][k, q] = 1 if q >= 128*r + k  (within a 512-q span, k-block offset r)
    qidx = np.arange(SPAN)[None, None, :]
    kidx = np.arange(P)[:, None, None]
    ridx = np.arange(4)[None, :, None]
    maskT = (qidx >= P * ridx + kidx).astype(bft)  # (128, 4, 512)
    ident = np.eye(P, dtype=np.float32).astype(bft)

    xTs = [np.ascontiguousarray(x[b].T).astype(bft) for b in range(2)]
    wq16 = wq.astype(bft)
    wk16 = wk.astype(bft)
    wv16 = wv.astype(bft)
    wo16 = wo.astype(bft)
    in_maps = []
    for c in range(8):
        b, tp = divmod(c, 4)
        in_maps.append(
            {
                "xT": xTs[b],
                "wq": np.ascontiguousarray(wq16[:, tp * FQ : (tp + 1) * FQ]),
                "wk": np.ascontiguousarray(wk16[:, tp * FK : (tp + 1) * FK]),
                "wv": np.ascontiguousarray(wv16[:, tp * FK : (tp + 1) * FK]),
                "wo": np.ascontiguousarray(wo16[tp * FQ : (tp + 1) * FQ, :]),
                "cc": cc,
                "ss": ss,
                "maskT": maskT,
                "ident": ident,
            }
        )
    return in_maps


def kernel(x, cos, sin, wq, wk, wv, wo, trace=False):
    x = np.asarray(x, dtype=np.float32)
    cos = np.asarray(cos, dtype=np.float32)
    sin = np.asarray(sin, dtype=np.float32)
    wq = np.asarray(wq, dtype=np.float32)
    wk = np.asarray(wk, dtype=np.float32)
    wv = np.asarray(wv, dtype=np.float32)
    wo = np.asarray(wo, dtype=np.float32)

    nc = _get_nc()
    in_maps = _host_inputs(x, cos, sin, wq, wk, wv, wo)
    res = run_bass_kernel_spmd(nc, in_maps, core_ids=list(range(8)), trace=trace)
    out = np.zeros((2, T, C), dtype=np.float32)
    for c in range(8):
        b = c // 4
        out[b] += res.results[c]["y"]
    if trace:
        return out, res
    return out
